# revision 1
# baseline (speedup 1.0000x reference)
"""GroupedEmbeddingBag kernel for 8 trn2 NeuronCores.

Table-parallel: core c handles table c (weights[c], values[c], offsets[c]).
Per core: 1600 indirect-DMA gathers (128 rows of 512B each) pull embedding
rows into SBUF in position order; TensorE matmuls with host-baked 0/1
selection matrices segment-sum them into PSUM "epoch" tiles (one epoch = 4
tiles = 512 positions, bag window W slots). Epoch results stream back to
DRAM; the host maps (epoch, slot) -> bag and concatenates tables.
"""

import sys

sys.path.insert(0, "/opt/trn_rl_repo")

import numpy as np

T, V, D, B = 8, 100000, 128, 4096
L = 204800
P = 128
NTILES = L // P            # 1600
EP_TILES = 4               # tiles per epoch
NEP = NTILES // EP_TILES   # 400 epochs
CHUNK_TILES = 16           # sel streaming chunk (4 epochs)
NCHUNK = NTILES // CHUNK_TILES
OUT_RING_EP = 4            # epochs per output DMA

_compiled = {}


def _patch_drain(tile_mod, mybir):
    from concourse.vector_clock import ScopedClock

    def _patched(self, tick_clock, wait_clock):
        # this walrus build allows only ONE sync-wait on the tail Drain:
        # spread the rest over preceding nops, one wait each.
        NNOPS = 64
        nops = [self.nc.sync.nop(nofuse=True, hint=f"dw_{i}") for i in range(NNOPS)]
        drain_inst = self.nc.sync.drain()
        wait_clock.add_sem_waits(
            drain_inst.ins, ScopedClock({None: tick_clock.global_clock})
        )
        dsi = drain_inst.ins.sync_info
        waits = list(dsi.on_wait) if dsi else []
        if len(waits) > 1:
            del dsi.on_wait[1:]
            rest = waits[1:]
            assert len(rest) <= NNOPS, f"too many drain waits: {len(waits)}"
            for nop, w in zip(nops, rest):
                nsi = nop.ins.sync_info
                if nsi is None:
                    nop.ins.sync_info = mybir.SyncInfo(on_wait=[w], on_update=[])
                else:
                    nsi.on_wait.append(w)
        self.nc.all_engine_barrier()
        popped = self.nc._tile_sem_poison_stack.pop()
        assert popped is self._sem_poison
        self.nc.clear_and_free_semaphores(list(self.sems.allocated().values()))
        self.nc.all_engine_barrier()

    tile_mod.TileContext._drain_and_barrier = _patched


def _split_waits(nc, mybir, maxw=1):
    # this walrus build rejects >1 sync-wait on an instruction: hoist extra
    # waits onto same-engine nops spliced in directly before it.
    cnt = 0
    for fn in nc.m.functions:
        for blk in fn.blocks:
            new_insts = []
            for inst in blk.instructions:
                si = inst.sync_info
                if si is not None and len(si.on_wait) > maxw:
                    extra = list(si.on_wait[maxw:])
                    del si.on_wait[maxw:]
                    for w in extra:
                        nop = mybir.InstNoOp(
                            name=f"waitnop-{cnt}", engine=inst.engine, ins=[], outs=[]
                        )
                        cnt += 1
                        nop.sync_info = mybir.SyncInfo(on_wait=[w], on_update=[])
                        new_insts.append(nop)
                new_insts.append(inst)
            blk.instructions[:] = new_insts
    return cnt


def _build(W):
    import concourse.bass as bass
    import concourse.mybir as mybir
    import concourse.tile as tile

    _patch_drain(tile, mybir)

    nc = bass.Bass()
    wt = nc.declare_dram_parameter("wt", [V, D], mybir.dt.float32, isOutput=False)
    vals = nc.declare_dram_parameter("vals", [P, NTILES], mybir.dt.int32, isOutput=False)
    sel = nc.declare_dram_parameter("sel", [P, NTILES * W], mybir.dt.float32, isOutput=False)
    oslots = nc.declare_dram_parameter("oslots", [W, NEP * D], mybir.dt.float32, isOutput=True)

    with tile.TileContext(nc) as tc:
        with (
            tc.tile_pool(name="valsp", bufs=1) as valsp,
            tc.tile_pool(name="selp", bufs=3) as selp,
            tc.tile_pool(name="ep", bufs=8) as ep,
            tc.tile_pool(name="outp", bufs=2) as outp,
            tc.tile_pool(name="psum", bufs=8, space="PSUM") as psump,
        ):
            vals_sb = valsp.tile([P, NTILES], mybir.dt.int32)
            nc.sync.dma_start(out=vals_sb[:], in_=vals[:])
            out_ring = None
            psum_t = None
            for c in range(NCHUNK):
                sel_sb = selp.tile([P, CHUNK_TILES * W], mybir.dt.float32, tag="sel")
                nc.sync.dma_start(
                    out=sel_sb[:], in_=sel[:, c * CHUNK_TILES * W:(c + 1) * CHUNK_TILES * W]
                )
                for tl in range(CHUNK_TILES):
                    t = c * CHUNK_TILES + tl
                    e = t // EP_TILES
                    ph = t % EP_TILES
                    et = ep.tile([P, D], mybir.dt.float32, tag="e")
                    nc.gpsimd.indirect_dma_start(
                        out=et[:],
                        out_offset=None,
                        in_=wt[:],
                        in_offset=bass.IndirectOffsetOnAxis(
                            ap=vals_sb[:, t:t + 1], axis=0
                        ),
                    )
                    if ph == 0:
                        psum_t = psump.tile([W, D], mybir.dt.float32, tag="ps")
                    nc.tensor.matmul(
                        out=psum_t[:],
                        lhsT=sel_sb[:, tl * W:(tl + 1) * W],
                        rhs=et[:],
                        start=(ph == 0),
                        stop=(ph == EP_TILES - 1),
                    )
                    if ph == EP_TILES - 1:
                        er = e % OUT_RING_EP
                        if er == 0:
                            out_ring = outp.tile([W, OUT_RING_EP * D], mybir.dt.float32, tag="or")
                        nc.vector.tensor_copy(
                            out=out_ring[:, er * D:(er + 1) * D], in_=psum_t[:]
                        )
                        if er == OUT_RING_EP - 1:
                            e0 = e - (OUT_RING_EP - 1)
                            nc.sync.dma_start(
                                out=oslots[:, e0 * D:(e0 + OUT_RING_EP) * D],
                                in_=out_ring[:],
                            )
    _split_waits(nc, mybir)
    return nc


def kernel(values, offsets, weights):
    from concourse.bass_utils import run_bass_kernel_spmd

    values = np.asarray(values)
    offsets = np.asarray(offsets)
    weights = np.ascontiguousarray(np.asarray(weights, dtype=np.float32))

    pos = np.arange(L)
    seg = np.empty((T, L), dtype=np.int64)
    for c in range(T):
        seg[c] = np.searchsorted(offsets[c, 1:], pos, side="right")

    # epoch windows: epoch e covers positions [512e, 512e+512)
    segr = seg.reshape(T, NEP, EP_TILES * P)
    b_lo = segr[:, :, 0]                      # [T, NEP]
    b_hi = segr[:, :, -1]
    S = (b_hi - b_lo + 1).astype(np.int64)    # slots used per epoch
    W = int(S.max())
    W = max(4, (W + 3) // 4 * 4)
    assert W <= 128, f"epoch bag-window {W} exceeds PSUM partition limit"

    in_maps = []
    for c in range(T):
        vals_t = np.ascontiguousarray(
            values[c].reshape(NTILES, P).T.astype(np.int32)
        )  # [P, NTILES]
        # sel[j, t, s] = 1 if seg[128t + j] == b_lo[e(t)] + s
        seg_l = seg[c].reshape(NTILES, P).T.astype(np.int32)  # [P, NTILES]
        base = np.repeat(b_lo[c], EP_TILES).astype(np.int32)  # [NTILES]
        loc = seg_l - base[None, :]                           # [P, NTILES]
        selm = (loc[:, :, None] == np.arange(W, dtype=np.int32)[None, None, :])
        sel = np.ascontiguousarray(
            selm.reshape(P, NTILES * W).astype(np.float32)
        )
        in_maps.append({"wt": weights[c], "vals": vals_t, "sel": sel})

    key = W
    if key not in _compiled:
        _compiled.clear()
        _compiled[key] = _build(W)
    nc = _compiled[key]

    global _last_inmaps
    _last_inmaps = in_maps
    res = run_bass_kernel_spmd(nc, in_maps, core_ids=list(range(T)))

    out = np.zeros((B, T * D), dtype=np.float32)
    for c in range(T):
        osl = res.results[c]["oslots"].reshape(W, NEP, D)
        pooled = np.zeros((B, D), dtype=np.float32)
        for e in range(NEP):
            lo = int(b_lo[c, e])
            n = int(S[c, e])
            pooled[lo:lo + n] += osl[:n, e, :]
        out[:, c * D:(c + 1) * D] = pooled
    return out


if __name__ == "__main__":
    rng = np.random.default_rng(0)
    values = rng.integers(0, V, size=(T, L)).astype(np.int64)
    inner = np.sort(rng.integers(0, L, size=(T, B - 1)), axis=1)
    offsets = np.concatenate(
        [np.zeros((T, 1), np.int64), inner, np.full((T, 1), L, np.int64)], axis=1
    )
    weights = (rng.standard_normal((T, V, D)) * 0.01).astype(np.float32)
    out = kernel(values, offsets, weights)
    # numpy reference
    exp = np.zeros((B, T * D), dtype=np.float32)
    for c in range(T):
        pooled = np.zeros((B, D), np.float32)
        np.add.at(pooled, np.searchsorted(offsets[c, 1:], np.arange(L), side="right"), weights[c][values[c]])
        exp[:, c * D:(c + 1) * D] = pooled
    err = np.linalg.norm(out - exp) / np.linalg.norm(exp)
    print("self-check rel err:", err)

